# Initial kernel scaffold
#
"""Trainium2 Bass kernel for nn_MetaLayer_2551210573871 (dense_mlp).

Math:  out[b,o] = sum_i feature[b,i] * ((signal @ T_1).reshape(B,I,O)[b,i,o] + M_1[i,o])
             = sum_{s,i} signal[b,s]*feature[b,i]*T_1[s,i,o]  +  (feature @ M_1)[b,o]

Strategy (data-parallel over batch, 8 cores, B_local = 512):
  For each s: G_s = feature_local @ T_1[s]     (PE, f32r, s-pairs -> N=512 matmuls)
  out_local  = sum_s signal_local[:, s] * G_s + feature_local @ M_1
The per-example weighted accumulation (stage B) is elementwise work that
cannot run on the PE; it is split across DVE / ACT / GpSimd to keep all
engines balanced.  T_1 is pre-transposed on the host to [i, s, o] so each
SBUF load is one large fully-contiguous DMA (1 MiB, 8 s-values).
"""
import numpy as np

import concourse.bacc as bacc
import concourse.mybir as mybir
import concourse.tile as tile
from concourse.bass_utils import run_bass_kernel_spmd
from concourse.masks import make_identity

S_DIM, IN_DIM, OUT_DIM, BATCH = 128, 256, 256, 4096
N_CORES = 8
BL = BATCH // N_CORES          # 512 examples per core
NBT = BL // 128                # 4 batch tiles of 128
S_PER_OCT = 8                  # s-values per T1 DMA tile
NOCT = S_DIM // S_PER_OCT      # 16
NPAIR_PER_OCT = S_PER_OCT // 2

F32 = mybir.dt.float32
F32R = mybir.dt.float32r

# stage-B path mix per (pair, bt) unit, LP-optimal for measured op costs:
#   dve:      2x DVE scalar_tensor_tensor from PSUM   (DVE 519ns/op)
#   copy_stt: ACT pair-copy PSUM->SBUF (720ns) + 2x DVE STT from SBUF (396ns)
#   act_gps:  2x [ACT scaled-copy (563ns) + GpSimd add (691ns)]
_PATH_FRACS = {"dve": 0.42, "copy_stt": 0.20, "act_gps": 0.38}


def _make_assignment(n_units):
    used = {k: 0 for k in _PATH_FRACS}
    out = []
    for u in range(n_units):
        best, best_def = None, None
        for k, f in _PATH_FRACS.items():
            deficit = f * (u + 1) - used[k]
            if best_def is None or deficit > best_def:
                best, best_def = k, deficit
        used[best] += 1
        out.append(best)
    # drain bias: slowest-chain path (act_gps) must not land at the very end
    for u in range(len(out) - 16, len(out)):
        if out[u] == "act_gps":
            out[u] = "copy_stt" if u < len(out) - 6 else "dve"
    return out


def _build():
    nc = bacc.Bacc("TRN2", target_bir_lowering=False, debug=False, num_devices=N_CORES)

    sig_d = nc.dram_tensor("signal", [BL, S_DIM], F32, kind="ExternalInput")
    feat_d = nc.dram_tensor("feature", [BL, IN_DIM], F32, kind="ExternalInput")
    # T_1 pre-transposed on host to [i, s*o]
    t1_d = nc.dram_tensor("T_1t", [IN_DIM, S_DIM * OUT_DIM], F32R, kind="ExternalInput")
    m1_d = nc.dram_tensor("M_1", [IN_DIM, OUT_DIM], F32R, kind="ExternalInput")
    out_d = nc.dram_tensor("out", [BL, OUT_DIM], F32, kind="ExternalOutput")

    OCT_COLS = S_PER_OCT * OUT_DIM  # 2048

    with tile.TileContext(nc) as tc:
        assignment = _make_assignment(NOCT * NPAIR_PER_OCT * NBT)
        with (
            tc.tile_pool(name="const", bufs=1) as const,
            tc.tile_pool(name="t1", bufs=20) as t1_pool,
            tc.tile_pool(name="tmp", bufs=10) as tmp_pool,
            tc.tile_pool(name="gsb", bufs=6) as gsb_pool,
            tc.tile_pool(name="psum", bufs=8, space="PSUM") as psum,
        ):
            ident = const.tile([128, 128], F32, tag="ident", name="ident")
            make_identity(nc, ident[:])

            # --- load per-core inputs ---
            sig = []
            for bt in range(NBT):
                t = const.tile([128, S_DIM], F32, tag=f"sig{bt}", name=f"sig{bt}")
                nc.sync.dma_start(out=t[:], in_=sig_d[bt * 128:(bt + 1) * 128, :])
                sig.append(t)

            feat = []
            for bt in range(NBT):
                t = const.tile([128, IN_DIM], F32, tag=f"feat{bt}", name=f"feat{bt}")
                nc.sync.dma_start(out=t[:], in_=feat_d[bt * 128:(bt + 1) * 128, :])
                feat.append(t)

            m1 = []
            for ic in range(2):
                t = const.tile([128, OUT_DIM], F32R, tag=f"m1_{ic}", name=f"m1_{ic}")
                nc.sync.dma_start(out=t[:], in_=m1_d[ic * 128:(ic + 1) * 128, :])
                m1.append(t)

            # --- featT[ic][i, b] = feature_local^T, via PE transposes ---
            featT = [const.tile([128, BL], F32R, tag=f"featT{ic}", name=f"featT{ic}")
                     for ic in range(2)]
            for bt in range(NBT):
                for ic in range(2):
                    ps = psum.tile([128, 512], F32, tag="G", name="ps")
                    nc.tensor.transpose(
                        ps[:, 0:128], feat[bt][:, ic * 128:(ic + 1) * 128], ident[:]
                    )
                    nc.vector.tensor_copy(
                        featT[ic][:, bt * 128:(bt + 1) * 128], ps[:, 0:128]
                    )

            # --- acc[bt] = feature_local @ M_1  (the +M_1 term) ---
            acc = [const.tile([128, OUT_DIM], F32, tag=f"acc{bt}", name=f"acc{bt}")
                   for bt in range(NBT)]
            for bt in range(NBT):
                ps = psum.tile([128, 512], F32, tag="G", name="ps")
                for ic in range(2):
                    nc.tensor.matmul(
                        ps[:, 0:OUT_DIM],
                        featT[ic][:, bt * 128:(bt + 1) * 128],
                        m1[ic][:],
                        start=(ic == 0),
                        stop=(ic == 1),
                    )
                nc.vector.tensor_copy(acc[bt][:], ps[:, 0:OUT_DIM])

            # --- main loop: 64 s-pairs x 4 batch-tiles ---
            NPAIR = S_DIM // 2
            for p in range(NPAIR):
                t1t = []
                for ic in range(2):
                    t = t1_pool.tile([128, 512], F32R, tag="t1", name="t1t")
                    nc.sync.dma_start(
                        out=t[:],
                        in_=t1_d[ic * 128:(ic + 1) * 128, p * 512:(p + 1) * 512],
                    )
                    t1t.append(t)

                if True:
                    pl = 0
                    s0 = 2 * p
                    s1 = s0 + 1
                    for bt in range(NBT):
                        g = psum.tile([128, 512], F32, tag="G", name="g")
                        for ic in range(2):
                            nc.tensor.matmul(
                                g[:],
                                featT[ic][:, bt * 128:(bt + 1) * 128],
                                t1t[ic][:],
                                start=(ic == 0),
                                stop=(ic == 1),
                            )
                        # stage B: acc[bt] += sig[:, s] * G_s  (s0 then s1)
                        unit = p * NBT + bt
                        mode = assignment[unit]
                        if mode == "copy_stt":
                            gsb = gsb_pool.tile([128, 512], F32,
                                                tag="gsb", name="gsb")
                            nc.scalar.copy(gsb[:], g[:])
                            g_src = gsb
                        else:
                            g_src = g
                        for half, s in ((0, s0), (1, s1)):
                            g_half = g_src[:, half * OUT_DIM:(half + 1) * OUT_DIM]
                            s_col = sig[bt][:, s:s + 1]
                            if mode in ("dve", "copy_stt"):
                                nc.vector.scalar_tensor_tensor(
                                    acc[bt][:], g_half, s_col, acc[bt][:],
                                    mybir.AluOpType.mult, mybir.AluOpType.add,
                                )
                            else:
                                tmp = tmp_pool.tile([128, OUT_DIM], F32,
                                                    tag=f"tmp{bt}", name=f"tmp{bt}")
                                nc.scalar.activation(
                                    tmp[:], g_half,
                                    mybir.ActivationFunctionType.Identity,
                                    scale=s_col,
                                )
                                nc.gpsimd.tensor_tensor(
                                    acc[bt][:], tmp[:], acc[bt][:],
                                    mybir.AluOpType.add,
                                )

            for bt in range(NBT):
                nc.sync.dma_start(
                    out=out_d[bt * 128:(bt + 1) * 128, :], in_=acc[bt][:]
                )

    nc.compile()
    return nc


_cached = None


def make_in_maps(signal, feature, T_1, M_1):
    signal = np.ascontiguousarray(np.asarray(signal, dtype=np.float32))
    feature = np.ascontiguousarray(np.asarray(feature, dtype=np.float32))
    M_1 = np.ascontiguousarray(np.asarray(M_1, dtype=np.float32))
    # host transpose: [s, i*O+o] -> [i, s*O+o]
    T_1t = np.ascontiguousarray(
        np.asarray(T_1, dtype=np.float32)
        .reshape(S_DIM, IN_DIM, OUT_DIM)
        .transpose(1, 0, 2)
        .reshape(IN_DIM, S_DIM * OUT_DIM)
    )
    in_maps = []
    for c in range(N_CORES):
        sl = slice(c * BL, (c + 1) * BL)
        in_maps.append({
            "signal": signal[sl],
            "feature": feature[sl],
            "T_1t": T_1t,
            "M_1": M_1,
        })
    return in_maps


def kernel(signal, feature, T_1, M_1):
    global _cached
    if _cached is None:
        _cached = _build()
    nc = _cached
    in_maps = make_in_maps(signal, feature, T_1, M_1)
    res = run_bass_kernel_spmd(nc, in_maps, list(range(N_CORES))).results
    return np.concatenate([res[c]["out"] for c in range(N_CORES)], axis=0)



# revision 1
# speedup vs baseline: 1.4339x; 1.4339x over previous
"""Trainium2 Bass kernel for nn_MetaLayer_2551210573871 (dense_mlp).

Math:  out[b,o] = sum_i feature[b,i] * ((signal @ T_1).reshape(B,I,O)[b,i,o] + M_1[i,o])
             = sum_{s,i} signal[b,s]*feature[b,i]*T_1[s,i,o]  +  (feature @ M_1)[b,o]

Strategy (data-parallel over batch, 8 cores, B_local = 512):
  For each s: G_s = feature_local @ T_1[s]     (PE, f32r, s-pairs -> N=512 matmuls)
  out_local  = sum_s signal_local[:, s] * G_s + feature_local @ M_1
The per-example weighted accumulation (stage B) is elementwise work that
cannot run on the PE; it is split across DVE / ACT / GpSimd to keep all
engines balanced.  T_1 is pre-transposed on the host to [i, s, o] so each
SBUF load is one large fully-contiguous DMA (1 MiB, 8 s-values).
"""
import numpy as np

import concourse.bacc as bacc
import concourse.mybir as mybir
import concourse.tile as tile
from concourse.bass_utils import run_bass_kernel_spmd
from concourse.masks import make_identity

S_DIM, IN_DIM, OUT_DIM, BATCH = 128, 256, 256, 4096
N_CORES = 8
BL = BATCH // N_CORES          # 512 examples per core
NBT = BL // 128                # 4 batch tiles of 128
S_PER_OCT = 8                  # s-values per T1 DMA tile
NOCT = S_DIM // S_PER_OCT      # 16
NPAIR_PER_OCT = S_PER_OCT // 2

F32 = mybir.dt.float32
F32R = mybir.dt.float32r

# stage-B path mix per (pair, bt) unit, LP-optimal for measured op costs:
#   dve:      2x DVE scalar_tensor_tensor from PSUM   (DVE 519ns/op)
#   copy_stt: ACT pair-copy PSUM->SBUF (720ns) + 2x DVE STT from SBUF (396ns)
#   act_gps:  2x [ACT scaled-copy (563ns) + GpSimd add (691ns)]
_PATH_FRACS = {"dve": 0.42, "copy_stt": 0.20, "act_gps": 0.38}


def _make_assignment(n_units):
    used = {k: 0 for k in _PATH_FRACS}
    out = []
    for u in range(n_units):
        best, best_def = None, None
        for k, f in _PATH_FRACS.items():
            deficit = f * (u + 1) - used[k]
            if best_def is None or deficit > best_def:
                best, best_def = k, deficit
        used[best] += 1
        out.append(best)
    # drain bias: slowest-chain path (act_gps) must not land at the very end
    for u in range(len(out) - 16, len(out)):
        if out[u] == "act_gps":
            out[u] = "copy_stt" if u < len(out) - 6 else "dve"
    return out


def _build():
    nc = bacc.Bacc("TRN2", target_bir_lowering=False, debug=False, num_devices=N_CORES)

    sig_d = nc.dram_tensor("signal", [BL, S_DIM], F32, kind="ExternalInput")
    feat_d = nc.dram_tensor("feature", [BL, IN_DIM], F32, kind="ExternalInput")
    # T_1 pre-transposed on host to [i, s*o]
    t1_d = nc.dram_tensor("T_1t", [IN_DIM, S_DIM * OUT_DIM], F32R, kind="ExternalInput")
    m1_d = nc.dram_tensor("M_1", [IN_DIM, OUT_DIM], F32R, kind="ExternalInput")
    out_d = nc.dram_tensor("out", [BL, OUT_DIM], F32, kind="ExternalOutput")

    OCT_COLS = S_PER_OCT * OUT_DIM  # 2048

    with tile.TileContext(nc) as tc:
        assignment = _make_assignment(NOCT * NPAIR_PER_OCT * NBT)
        with (
            tc.tile_pool(name="const", bufs=1) as const,
            tc.tile_pool(name="t1", bufs=20) as t1_pool,
            tc.tile_pool(name="tmp", bufs=10) as tmp_pool,
            tc.tile_pool(name="gsb", bufs=6) as gsb_pool,
            tc.tile_pool(name="psum", bufs=8, space="PSUM") as psum,
        ):
            ident = const.tile([128, 128], F32, tag="ident", name="ident")
            make_identity(nc, ident[:])

            # --- load per-core inputs ---
            sig = []
            for bt in range(NBT):
                t = const.tile([128, S_DIM], F32, tag=f"sig{bt}", name=f"sig{bt}")
                nc.sync.dma_start(out=t[:], in_=sig_d[bt * 128:(bt + 1) * 128, :])
                sig.append(t)

            feat = []
            for bt in range(NBT):
                t = const.tile([128, IN_DIM], F32, tag=f"feat{bt}", name=f"feat{bt}")
                nc.sync.dma_start(out=t[:], in_=feat_d[bt * 128:(bt + 1) * 128, :])
                feat.append(t)

            m1 = []
            for ic in range(2):
                t = const.tile([128, OUT_DIM], F32R, tag=f"m1_{ic}", name=f"m1_{ic}")
                nc.sync.dma_start(out=t[:], in_=m1_d[ic * 128:(ic + 1) * 128, :])
                m1.append(t)

            # --- featT[ic][i, b] = feature_local^T, via PE transposes ---
            featT = [const.tile([128, BL], F32R, tag=f"featT{ic}", name=f"featT{ic}")
                     for ic in range(2)]
            for bt in range(NBT):
                for ic in range(2):
                    ps = psum.tile([128, 512], F32, tag="G", name="ps")
                    nc.tensor.transpose(
                        ps[:, 0:128], feat[bt][:, ic * 128:(ic + 1) * 128], ident[:]
                    )
                    nc.vector.tensor_copy(
                        featT[ic][:, bt * 128:(bt + 1) * 128], ps[:, 0:128]
                    )

            # --- acc[bt] = feature_local @ M_1  (the +M_1 term) ---
            acc = [const.tile([128, OUT_DIM], F32, tag=f"acc{bt}", name=f"acc{bt}")
                   for bt in range(NBT)]
            for bt in range(NBT):
                ps = psum.tile([128, 512], F32, tag="G", name="ps")
                for ic in range(2):
                    nc.tensor.matmul(
                        ps[:, 0:OUT_DIM],
                        featT[ic][:, bt * 128:(bt + 1) * 128],
                        m1[ic][:],
                        start=(ic == 0),
                        stop=(ic == 1),
                    )
                nc.vector.tensor_copy(acc[bt][:], ps[:, 0:OUT_DIM])

            # --- main loop: 64 s-pairs x 4 batch-tiles ---
            NPAIR = S_DIM // 2
            for p in range(NPAIR):
                t1t = []
                for ic in range(2):
                    t = t1_pool.tile([128, 512], F32R, tag="t1", name="t1t")
                    nc.sync.dma_start(
                        out=t[:],
                        in_=t1_d[ic * 128:(ic + 1) * 128, p * 512:(p + 1) * 512],
                    )
                    t1t.append(t)

                if True:
                    pl = 0
                    s0 = 2 * p
                    s1 = s0 + 1
                    for bt in range(NBT):
                        g = psum.tile([128, 512], F32, tag="G", name="g")
                        for ic in range(2):
                            nc.tensor.matmul(
                                g[:],
                                featT[ic][:, bt * 128:(bt + 1) * 128],
                                t1t[ic][:],
                                start=(ic == 0),
                                stop=(ic == 1),
                            )
                        # stage B: acc[bt] += sig[:, s] * G_s  (s0 then s1)
                        unit = p * NBT + bt
                        mode = assignment[unit]
                        if mode == "copy_stt":
                            gsb = gsb_pool.tile([128, 512], F32,
                                                tag="gsb", name="gsb")
                            nc.scalar.copy(gsb[:], g[:])
                            g_src = gsb
                        else:
                            g_src = g
                        for half, s in ((0, s0), (1, s1)):
                            g_half = g_src[:, half * OUT_DIM:(half + 1) * OUT_DIM]
                            s_col = sig[bt][:, s:s + 1]
                            if mode in ("dve", "copy_stt"):
                                nc.vector.scalar_tensor_tensor(
                                    acc[bt][:], g_half, s_col, acc[bt][:],
                                    mybir.AluOpType.mult, mybir.AluOpType.add,
                                )
                            else:
                                tmp = tmp_pool.tile([128, OUT_DIM], F32,
                                                    tag=f"tmp{bt}", name=f"tmp{bt}")
                                nc.scalar.activation(
                                    tmp[:], g_half,
                                    mybir.ActivationFunctionType.Identity,
                                    scale=s_col,
                                )
                                nc.gpsimd.tensor_tensor(
                                    acc[bt][:], tmp[:], acc[bt][:],
                                    mybir.AluOpType.add,
                                )

            for bt in range(NBT):
                nc.sync.dma_start(
                    out=out_d[bt * 128:(bt + 1) * 128, :], in_=acc[bt][:]
                )

    nc.compile()
    return nc


_cached = None


def make_in_maps(signal, feature, T_1, M_1):
    signal = np.ascontiguousarray(np.asarray(signal, dtype=np.float32))
    feature = np.ascontiguousarray(np.asarray(feature, dtype=np.float32))
    M_1 = np.ascontiguousarray(np.asarray(M_1, dtype=np.float32))
    # host transpose: [s, i*O+o] -> [i, s*O+o]
    T_1t = np.ascontiguousarray(
        np.asarray(T_1, dtype=np.float32)
        .reshape(S_DIM, IN_DIM, OUT_DIM)
        .transpose(1, 0, 2)
        .reshape(IN_DIM, S_DIM * OUT_DIM)
    )
    in_maps = []
    for c in range(N_CORES):
        sl = slice(c * BL, (c + 1) * BL)
        in_maps.append({
            "signal": signal[sl],
            "feature": feature[sl],
            "T_1t": T_1t,
            "M_1": M_1,
        })
    return in_maps


def kernel(signal, feature, T_1, M_1):
    global _cached
    if _cached is None:
        _cached = _build()
    nc = _cached
    in_maps = make_in_maps(signal, feature, T_1, M_1)
    res = run_bass_kernel_spmd(nc, in_maps, list(range(N_CORES))).results
    return np.concatenate([res[c]["out"] for c in range(N_CORES)], axis=0)

